# revision 1
# baseline (speedup 1.0000x reference)
"""Trainium2 Bass kernel for nn_AtomsGPT (GPT-2-style dense transformer).

B=4, T=1024, D=1024, H=16 heads, L=8 layers, V=50257, tied LM head.

Sharding (8 NeuronCores):
- Token-data-parallel trunk: core c owns batch c//2, pair-rank r=c%2.
  Rank r takes the even (r=0) / odd (r=1) 128-position tiles of the
  sequence, interleaved for causal-attention load balance.
- Per layer, the pair exchanges LN1 outputs via a 2-rank AllGather and
  each core computes K/V for all 1024 positions of its batch (the extra
  K/V matmul is cheaper than exchanging K/V and overlaps the collective).
- The tied LM head is sharded over vocab (6288 padded columns per core)
  after an 8-rank AllGather of the final layernormed activations.

All matmuls run in bf16 with fp32 PSUM accumulation; the residual stream
and layernorm statistics stay fp32. LN scales and the attention scale
are folded into weight matrices on the host (exact); all bias vectors in
this problem are structurally zero (asserted).
"""

import sys

for _p in ("/opt/trn_rl_repo", "/root/.axon_site"):
    if _p not in sys.path:
        sys.path.insert(0, _p)

import numpy as np
import ml_dtypes

import concourse.bass as bass
import concourse.tile as tile
from concourse import bacc, mybir
from concourse.bass_utils import run_bass_kernel_spmd

F32 = mybir.dt.float32
BF16 = mybir.dt.bfloat16
AF = mybir.ActivationFunctionType
OP = mybir.AluOpType

B, T, D, H, L, V = 4, 1024, 1024, 16, 8, 50257
HD = D // H  # 64
EPS = 1e-5
N_CORES = 8
TOK = 512           # tokens per core
P = 128
VP = 6288           # per-core padded vocab slice (8*6288 = 50304 >= V)
PAIRS = [[0, 1], [2, 3], [4, 5], [6, 7]]
WORLD = [list(range(N_CORES))]


def positions_for_rank(r):
    """Global positions owned by pair-rank r, in local order (increasing)."""
    tiles = [2 * j + r for j in range(4)]
    return np.concatenate([np.arange(128 * t, 128 * (t + 1)) for t in tiles])


def _ln_tm(nc, sb, stat, psT, x_ap, xn_fm, ident, eng_evac):
    """LayerNorm of token-major x_ap [128, 4, 1024] f32 -> feature-major
    bf16 xn_fm [128, 8, 512].  Scale/bias are folded into downstream
    weights on the host, so this computes plain (x - mean) * rsqrt(var)."""
    ssum = stat.tile([128, 4], F32, tag="ssum")
    ssq = stat.tile([128, 4], F32, tag="ssq")
    for t in range(4):
        nc.vector.reduce_sum(ssum[:, t : t + 1], x_ap[:, t, :], axis=mybir.AxisListType.X)
        sc = sb.tile([128, 1024], F32, tag="ln_sc")
        nc.scalar.activation(sc[:], x_ap[:, t, :], AF.Square,
                             accum_out=ssq[:, t : t + 1])
    m = stat.tile([128, 4], F32, tag="m")
    nc.vector.tensor_scalar_mul(m[:], ssum[:], 1.0 / D)
    var = stat.tile([128, 4], F32, tag="var")
    nc.vector.tensor_scalar_mul(var[:], ssq[:], 1.0 / D)
    mm = stat.tile([128, 4], F32, tag="mm")
    nc.vector.tensor_mul(mm[:], m[:], m[:])
    nc.vector.tensor_sub(var[:], var[:], mm[:])
    eps = stat.tile([128, 1], F32, tag="eps")
    nc.vector.memset(eps[:], EPS)
    std = stat.tile([128, 4], F32, tag="std")
    nc.scalar.activation(std[:], var[:], AF.Sqrt, bias=eps[:])
    rstd = stat.tile([128, 4], F32, tag="rstd")
    nc.vector.reciprocal(rstd[:], std[:])
    nmr = stat.tile([128, 4], F32, tag="nmr")
    nc.vector.tensor_mul(nmr[:], m[:], rstd[:])
    nc.vector.tensor_scalar_mul(nmr[:], nmr[:], -1.0)
    for t in range(4):
        xn = sb.tile([128, 1024], BF16, tag="ln_xn")
        nc.vector.tensor_scalar(xn[:], x_ap[:, t, :], rstd[:, t : t + 1],
                                nmr[:, t : t + 1], OP.mult, OP.add)
        ptr = psT.tile([128, 8, 128], BF16, tag="tr")
        for kk in range(8):
            nc.tensor.transpose(ptr[:, kk, :], xn[:, kk * 128:(kk + 1) * 128], ident[:])
        eng = nc.vector if (eng_evac + t) % 2 == 0 else nc.scalar
        if eng is nc.vector:
            nc.vector.tensor_copy(xn_fm[:, :, t * 128:(t + 1) * 128], ptr[:])
        else:
            nc.scalar.copy(xn_fm[:, :, t * 128:(t + 1) * 128], ptr[:])


def build(n_layers=L, dbg=False, gelu_sim=False, no_cc=False, stages=99):
    nc = bacc.Bacc("TRN2", target_bir_lowering=False, debug=False,
                   num_devices=N_CORES)

    x0_h = nc.dram_tensor("x0", [TOK, D], F32, kind="ExternalInput")
    wqkv_h = nc.dram_tensor("wqkv", [n_layers, D, 3 * D], BF16, kind="ExternalInput")
    wp_h = nc.dram_tensor("wp", [n_layers, D, D], BF16, kind="ExternalInput")
    w1_h = nc.dram_tensor("w1", [n_layers, D, 4 * D], BF16, kind="ExternalInput")
    w2_h = nc.dram_tensor("w2", [n_layers, 4 * D, D], BF16, kind="ExternalInput")
    embT_h = nc.dram_tensor("embT", [D, VP], BF16, kind="ExternalInput")
    msk_h = nc.dram_tensor("msk", [2, 128, 128], BF16, kind="ExternalInput")
    ident_h = nc.dram_tensor("identin", [128, 128], BF16, kind="ExternalInput")
    ones64_h = nc.dram_tensor("ones64", [1, 64], BF16, kind="ExternalInput")
    out_h = nc.dram_tensor("out", [N_CORES * TOK, VP], F32, kind="ExternalOutput")

    dbg_outs = {}

    def dbg_dump(name, ap, shape, rearr=None):
        if not dbg:
            return
        t = nc.dram_tensor(f"dbg_{name}", list(shape), ap.dtype, kind="ExternalOutput")
        dst = t.ap() if rearr is None else t.ap().rearrange(rearr)
        nc.sync.dma_start(dst, ap)
        dbg_outs[name] = shape

    ag_in = [nc.dram_tensor(f"agin{l}", [D, TOK], BF16, kind="Internal")
             for l in range(n_layers)]
    ag_out = [nc.dram_tensor(f"agout{l}", [2 * D, TOK], BF16, kind="Internal")
              for l in range(n_layers)]
    agf_in = nc.dram_tensor("agfin", [D, TOK], BF16, kind="Internal")
    agf_out = nc.dram_tensor("agfout", [N_CORES * D, TOK], BF16, kind="Internal",
                             addr_space="Shared")

    with tile.TileContext(nc) as tc:
      with tc.tile_pool(name="const", bufs=1) as constp, \
           tc.tile_pool(name="xres", bufs=1) as xresp:
        with tc.tile_pool(name="stat", bufs=2) as stat, \
             tc.tile_pool(name="sb", bufs=2) as sb, \
             tc.tile_pool(name="act", bufs=1) as actp, \
             tc.tile_pool(name="wch", bufs=4) as wch, \
             tc.tile_pool(name="pp", bufs=6) as pp, \
             tc.tile_pool(name="psA", bufs=4, space="PSUM") as psA, \
             tc.tile_pool(name="psB", bufs=3, space="PSUM") as psB, \
             tc.tile_pool(name="psT", bufs=1, space="PSUM") as psT:

            ident = constp.tile([128, 128], BF16)
            nc.sync.dma_start(ident[:], ident_h[:])
            msk = constp.tile([128, 2, 128], BF16)
            nc.sync.dma_start(msk[:], msk_h.ap().rearrange("b p q -> p b q"))
            ones64 = constp.tile([1, 64], BF16)
            nc.sync.dma_start(ones64[:], ones64_h[:])

            # residual stream, token-major fp32 [part, tok-tile, D]
            x = xresp.tile([128, 4, D], F32)
            nc.sync.dma_start(x[:], x0_h.ap().rearrange("(t p) d -> p t d", p=128))

            for l in range(n_layers):
                # ---- LN1 -> xn_fm (feature-major bf16), kick pair AllGather
                xn_fm = actp.tile([128, 8, TOK], BF16, tag="xn_fm")
                _ln_tm(nc, sb, stat, psT, x, xn_fm, ident, eng_evac=0)
                nc.sync.dma_start(
                    ag_in[l].ap().rearrange("(kk p) t -> p kk t", p=128), xn_fm[:])
                if no_cc:
                    nc.sync.dma_start(ag_out[l][0:D, :], ag_in[l][:])
                    nc.sync.dma_start(ag_out[l][D:2 * D, :], ag_in[l][:])
                else:
                    nc.gpsimd.collective_compute(
                        "AllGather", OP.bypass, replica_groups=PAIRS,
                        ins=[ag_in[l][:]], outs=[ag_out[l][:]])
                if l == 0:
                    dbg_dump("xn_fm0", xn_fm[:], [128, 8, TOK])

                # ---- Q^T (feature-major) from local xn_fm
                if stages < 3:
                    continue
                q_fm = actp.tile([128, 8, TOK], BF16, tag="q_fm")
                for ch in range(2):
                    wt = wch.tile([128, 8, 512], BF16, tag="w", name=f"wq{l}_{ch}")
                    nc.sync.dma_start(
                        wt[:], wqkv_h[l, :, ch * 512:(ch + 1) * 512].rearrange(
                            "(kk p) c -> p kk c", p=128))
                    for mi in range(4):
                        ps = psA.tile([128, TOK], F32, tag="mm")
                        for kk in range(8):
                            nc.tensor.matmul(ps[:], wt[:, kk, mi * 128:(mi + 1) * 128],
                                             xn_fm[:, kk, :], start=(kk == 0), stop=(kk == 7))
                        nc.scalar.copy(q_fm[:, ch * 4 + mi, :], ps[:])

                # ---- gathered xn (both ranks) from the AllGather
                if stages < 4:
                    continue
                xn_src = actp.tile([128, 16, TOK], BF16, tag="xn_src")
                nc.sync.dma_start(
                    xn_src[:],
                    ag_out[l].ap().rearrange("(b kk p) t -> p (b kk) t", b=2, p=128))

                # ---- K^T (feature-major) for all 1024 positions
                k_all = actp.tile([128, 16, TOK], BF16, tag="k_all")
                for ch in range(2):
                    wt = wch.tile([128, 8, 512], BF16, tag="w", name=f"wk{l}_{ch}")
                    nc.sync.dma_start(
                        wt[:], wqkv_h[l, :, D + ch * 512:D + (ch + 1) * 512].rearrange(
                            "(kk p) c -> p kk c", p=128))
                    for b in range(2):
                        for mi in range(4):
                            ps = psA.tile([128, TOK], F32, tag="mm")
                            for kk in range(8):
                                nc.tensor.matmul(
                                    ps[:], wt[:, kk, mi * 128:(mi + 1) * 128],
                                    xn_src[:, b * 8 + kk, :], start=(kk == 0), stop=(kk == 7))
                            nc.vector.tensor_copy(k_all[:, b * 8 + ch * 4 + mi, :], ps[:])

                # ---- V (token-major) for all positions, with ones column
                if stages < 5:
                    continue
                v_all = actp.tile([128, 8, H, HD + 1], BF16, tag="v_all")
                nc.vector.memset(v_all[:, :, :, HD:HD + 1], 1.0)
                for ch in range(2):
                    wt = wch.tile([128, 8, 512], BF16, tag="w", name=f"wv{l}_{ch}")
                    nc.sync.dma_start(
                        wt[:], wqkv_h[l, :, 2 * D + ch * 512:2 * D + (ch + 1) * 512].rearrange(
                            "(kk p) c -> p kk c", p=128))
                    for b in range(2):
                        for t in range(4):
                            ps = psA.tile([128, TOK], F32, tag="mm")
                            for kk in range(8):
                                nc.tensor.matmul(
                                    ps[:], xn_src[:, b * 8 + kk, t * 128:(t + 1) * 128],
                                    wt[:, kk, :], start=(kk == 0), stop=(kk == 7))
                            nc.vector.tensor_copy(
                                v_all[:, b * 4 + t, ch * 8:(ch + 1) * 8, 0:HD],
                                ps[:].rearrange("p (h d) -> p h d", h=8))
                if l == 0:
                    dbg_dump("k_all0", k_all[:], [128, 16, TOK])
                    dbg_dump("v_all0", v_all[:], [128, 8, H, HD + 1])

                # ---- attention, head by head
                if stages < 6:
                    continue
                o_fm = actp.tile([128, 8, TOK], BF16, tag="o_fm")
                av_ps = {}
                for h in range(H):
                    po = (h % 2) * 64
                    kt = h // 2
                    avp = psB.tile([P, TOK], F32, tag="acc", name=f"av{l}_{h}")
                    av_ps[h] = avp
                    for b in range(2):
                        for i in range(4):
                            n = TOK - 128 * i
                            sp = psA.tile([128, n], F32, tag="mm")
                            nc.tensor.matmul(
                                sp[:],
                                k_all[po:po + 64, b * 8 + kt, i * 128:(i + 1) * 128],
                                q_fm[po:po + 64, kt, 128 * i:TOK],
                                start=True, stop=True)
                            pt = pp.tile([128, n], BF16, tag="p")
                            nc.scalar.activation(pt[:], sp[:], AF.Exp)
                            # causal mask on the diagonal 128-col block
                            nc.gpsimd.tensor_mul(pt[:, 0:128], pt[:, 0:128], msk[:, b, :])
                            nc.tensor.matmul(
                                avp[0:HD + 1, 128 * i:TOK],
                                v_all[:, b * 4 + i, h, :], pt[:],
                                start=(b == 0 and i == 0), stop=(b == 1 and i == 3))
                    if h % 2 == 1:
                        # denominators -> broadcast -> reciprocal -> scale o
                        den = sb.tile([1, 2, TOK], BF16, tag="den")
                        for hh in (h - 1, h):
                            nc.scalar.copy(den[0:1, hh % 2, :],
                                           av_ps[hh][HD:HD + 1, :])
                        bp = psA.tile([128, TOK], F32, tag="mm")
                        nc.tensor.matmul(bp[0:64, :], ones64[:], den[0:1, 0, :],
                                         start=True, stop=True)
                        nc.tensor.matmul(bp[64:128, :], ones64[:], den[0:1, 1, :],
                                         start=True, stop=True)
                        rb = sb.tile([128, TOK], F32, tag="rb")
                        nc.vector.reciprocal(rb[:], bp[:])
                        nc.vector.tensor_tensor(o_fm[0:64, kt, :], av_ps[h - 1][0:HD, :],
                                                rb[0:64, :], OP.mult)
                        nc.vector.tensor_tensor(o_fm[64:128, kt, :], av_ps[h][0:HD, :],
                                                rb[64:128, :], OP.mult)
                        del av_ps[h - 1], av_ps[h]
                if l == 0:
                    dbg_dump("o_fm0", o_fm[:], [128, 8, TOK])

                # ---- projection (token-major) + residual
                if stages < 7:
                    continue
                for ch in range(2):
                    wt = wch.tile([128, 8, 512], BF16, tag="w", name=f"wpj{l}_{ch}")
                    nc.sync.dma_start(
                        wt[:], wp_h[l, :, ch * 512:(ch + 1) * 512].rearrange(
                            "(kk p) c -> p kk c", p=128))
                    for t in range(4):
                        ps = psA.tile([128, 512], F32, tag="mm")
                        for kk in range(8):
                            nc.tensor.matmul(
                                ps[:], o_fm[:, kk, t * 128:(t + 1) * 128],
                                wt[:, kk, :], start=(kk == 0), stop=(kk == 7))
                        nc.vector.tensor_add(x[:, t, ch * 512:(ch + 1) * 512],
                                             x[:, t, ch * 512:(ch + 1) * 512], ps[:])
                if l == 0:
                    dbg_dump("xattn0", x[:], [128, 4, D])

                # ---- LN2 -> xn2_fm
                if stages < 8:
                    continue
                xn2_fm = actp.tile([128, 8, TOK], BF16, tag="xn2_fm")
                _ln_tm(nc, sb, stat, psT, x, xn2_fm, ident, eng_evac=1)

                # ---- FFN: ff1 full-token, ff2 in token halves
                h_sb = actp.tile([128, 32, TOK], BF16, tag="h_sb")
                for mc in range(8):
                    wt = wch.tile([128, 8, 512], BF16, tag="w", name=f"w1_{l}_{mc}")
                    nc.sync.dma_start(
                        wt[:], w1_h[l, :, mc * 512:(mc + 1) * 512].rearrange(
                            "(kk p) c -> p kk c", p=128))
                    for mi in range(4):
                        ps = psA.tile([128, TOK], F32, tag="mm")
                        for kk in range(8):
                            nc.tensor.matmul(
                                ps[:], wt[:, kk, mi * 128:(mi + 1) * 128],
                                xn2_fm[:, kk, :], start=(kk == 0), stop=(kk == 7))
                        if gelu_sim:
                            gt = sb.tile([128, TOK], F32, tag="gelu_t")
                            nc.scalar.activation(gt[:], ps[:], AF.Sigmoid, scale=1.702)
                            nc.vector.tensor_tensor(h_sb[:, mc * 4 + mi, :], ps[:],
                                                    gt[:], OP.mult)
                        else:
                            nc.scalar.activation(h_sb[:, mc * 4 + mi, :], ps[:], AF.Gelu)
                for half in range(2):
                    for nch in range(2):
                        acc = [psB.tile([128, 512], F32, tag="acc",
                                        name=f"acc{l}_{half}_{nch}_{a}") for a in range(2)]
                        for kkc in range(8):
                            w2t = wch.tile([128, 4, 512], BF16, tag="w",
                                           name=f"w2_{l}_{half}_{nch}_{kkc}")
                            nc.sync.dma_start(
                                w2t[:],
                                w2_h[l, kkc * 512:(kkc + 1) * 512,
                                     nch * 512:(nch + 1) * 512].rearrange(
                                    "(kk p) c -> p kk c", p=128))
                            for kki in range(4):
                                for mi in range(2):
                                    nc.tensor.matmul(
                                        acc[mi][:],
                                        h_sb[:, kkc * 4 + kki,
                                             half * 256 + mi * 128:half * 256 + (mi + 1) * 128],
                                        w2t[:, kki, :],
                                        start=(kkc == 0 and kki == 0),
                                        stop=(kkc == 7 and kki == 3))
                        for mi in range(2):
                            t = half * 2 + mi
                            nc.vector.tensor_add(x[:, t, nch * 512:(nch + 1) * 512],
                                                 x[:, t, nch * 512:(nch + 1) * 512],
                                                 acc[mi][:])
                if l == 0:
                    dbg_dump("xlayer0", x[:], [128, 4, D])

            if stages < 9:
                # early-exit build for bisection: dump residual so work isn't DCE'd
                xdump = nc.dram_tensor("xdump", [128, 4, D], F32, kind="ExternalOutput")
                nc.sync.dma_start(xdump.ap(), x[:])

        # ---- final LN + LM head phase (separate pools; trunk SBUF released)
        with tc.tile_pool(name="stat2", bufs=2) as stat2, \
             tc.tile_pool(name="sb2", bufs=2) as sb2, \
             tc.tile_pool(name="hd", bufs=1) as hd, \
             tc.tile_pool(name="hout", bufs=4) as hout, \
             tc.tile_pool(name="psT2", bufs=1, space="PSUM") as psT2, \
             tc.tile_pool(name="psH", bufs=4, space="PSUM") as psH:
            if stages >= 9:
                from concourse.bass import _add_dep_helper
                xnf_fm = hd.tile([128, 8, TOK], BF16)
                _ln_tm(nc, sb2, stat2, psT2, x, xnf_fm, ident, eng_evac=0)
                agf_dma = nc.sync.dma_start(
                    agf_in.ap().rearrange("(kk p) t -> p kk t", p=128), xnf_fm[:])
                if no_cc:
                    for r_ in range(N_CORES):
                        nc.sync.dma_start(agf_out[r_ * D:(r_ + 1) * D, :], agf_in[:])
                else:
                    nc.gpsimd.collective_compute(
                        "AllGather", OP.bypass, replica_groups=WORLD,
                        ins=[agf_in[:]], outs=[agf_out[:]])
                if dbg:
                    dbg_dump("xnf_fm", xnf_fm[:], [128, 8, TOK])
                # stream the head weights in vocab chunks, deferred behind the
                # trunk (prevents the allocator grabbing 98KB/partition early)
                embT_sb = hd.tile([128, 8, VP], BF16)
                embT_src = embT_h.ap().rearrange("(kk p) v -> p kk v", p=128)
                for ci in range((VP + 511) // 512):
                    c0 = ci * 512
                    csz = min(512, VP - c0)
                    d = nc.sync.dma_start(embT_sb[:, :, c0:c0 + csz],
                                          embT_src[:, :, c0:c0 + csz])
                    _add_dep_helper(d.ins, agf_dma.ins, sync=True,
                                    reason="defer embT load behind trunk")
                xn_all = hd.tile([128, 64, TOK], BF16)
                xa_src = agf_out.ap().rearrange("(r kk p) t -> p (r kk) t", r=8, p=128)
                for r_ in range(8):
                    nc.sync.dma_start(xn_all[:, r_ * 8:(r_ + 1) * 8, :],
                                      xa_src[:, r_ * 8:(r_ + 1) * 8, :])
                nchunks = [(i * 512, min(512, VP - i * 512)) for i in range((VP + 511) // 512)]
                for mi in range(32):
                    r, t = mi // 4, mi % 4
                    for ni, (n0, nsz) in enumerate(nchunks):
                        ps = psH.tile([128, nsz], F32, tag="h")
                        for kk in range(8):
                            nc.tensor.matmul(
                                ps[:], xn_all[:, r * 8 + kk, t * 128:(t + 1) * 128],
                                embT_sb[:, kk, n0:n0 + nsz],
                                start=(kk == 0), stop=(kk == 7))
                        osb = hout.tile([128, nsz], F32, tag="o")
                        if ni % 2 == 0:
                            nc.vector.tensor_copy(osb[:], ps[:])
                        else:
                            nc.scalar.copy(osb[:], ps[:])
                        nc.sync.dma_start(out_h[mi * 128:(mi + 1) * 128, n0:n0 + nsz], osb[:])

    nc.compile()
    return nc, dbg_outs


def prepare_inputs(idx, tok_emb, pos_emb, qkv_w, qkv_b, proj_w, proj_b,
                   ff1_w, ff1_b, ff2_w, ff2_b, ln1_s, ln1_b, ln2_s, ln2_b,
                   lnf_s, lnf_b, n_layers=L):
    """Host-side sharding/folding. Returns per-core in_maps."""
    bf = ml_dtypes.bfloat16
    for name, v in (("qkv_b", qkv_b), ("proj_b", proj_b), ("ff1_b", ff1_b),
                    ("ff2_b", ff2_b), ("ln1_b", ln1_b), ("ln2_b", ln2_b),
                    ("lnf_b", lnf_b)):
        assert np.allclose(np.asarray(v), 0.0), f"nonzero {name} not supported"

    idx = np.asarray(idx)
    tok_emb = np.asarray(tok_emb, np.float32)
    pos_emb = np.asarray(pos_emb, np.float32)
    scale = 1.0 / np.sqrt(HD)

    # fold LN scales + attention scale into weights (exact)
    wqkv = (np.asarray(qkv_w[:n_layers], np.float32)
            * np.asarray(ln1_s[:n_layers], np.float32)[:, :, None]).copy()
    wqkv[:, :, :D] *= scale
    w1 = (np.asarray(ff1_w[:n_layers], np.float32)
          * np.asarray(ln2_s[:n_layers], np.float32)[:, :, None])
    wp = np.asarray(proj_w[:n_layers], np.float32)
    w2 = np.asarray(ff2_w[:n_layers], np.float32)
    embT_full = (tok_emb * np.asarray(lnf_s, np.float32)[None, :]).T  # [D, V]
    embT_pad = np.zeros((D, N_CORES * VP), np.float32)
    embT_pad[:, :V] = embT_full

    wqkv_b = wqkv.astype(bf)
    wp_b = wp.astype(bf)
    w1_b = w1.astype(bf)
    w2_b = w2.astype(bf)
    ident = np.eye(128, dtype=bf)
    ones64 = np.ones((1, 64), bf)

    tri = np.tril(np.ones((128, 128), np.float32)).T  # [kt, q] valid kt<=q
    msk_r = [np.zeros((2, 128, 128), np.float32) for _ in range(2)]
    msk_r[0][0] = tri          # even block diag: triangular
    msk_r[0][1] = 0.0          # odd block diag: fully masked
    msk_r[1][0] = 1.0          # even block diag: fully visible
    msk_r[1][1] = tri          # odd block diag: triangular

    in_maps = []
    for c in range(N_CORES):
        b, r = c // 2, c % 2
        pos = positions_for_rank(r)
        x0 = tok_emb[idx[b, pos]] + pos_emb[pos]
        in_maps.append({
            "x0": np.ascontiguousarray(x0, np.float32),
            "wqkv": wqkv_b, "wp": wp_b, "w1": w1_b, "w2": w2_b,
            "embT": np.ascontiguousarray(embT_pad[:, c * VP:(c + 1) * VP]).astype(bf),
            "msk": msk_r[r].astype(bf),
            "identin": ident,
            "ones64": ones64,
        })
    return in_maps


def assemble_output(results):
    """Per-core [4096, VP] f32 -> full logits [B, T, V] f32."""
    logits = np.empty((B, T, V), np.float32)
    pos_r = [positions_for_rank(0), positions_for_rank(1)]
    for c in range(N_CORES):
        out_c = results[c]["out"]  # [4096, VP]
        v0 = c * VP
        ncols = min(VP, V - v0)
        if ncols <= 0:
            continue
        for r in range(N_CORES):
            bb, rr = r // 2, r % 2
            logits[bb, pos_r[rr], v0:v0 + ncols] = \
                out_c[r * TOK:(r + 1) * TOK, :ncols]
    return logits


_NC_CACHE = {}


def _get_nc(n_layers=L, dbg=False):
    key = (n_layers, dbg)
    if key not in _NC_CACHE:
        _NC_CACHE[key] = build(n_layers=n_layers, dbg=dbg)
    return _NC_CACHE[key]


def kernel(**inputs):
    in_maps = prepare_inputs(**inputs)
    nc, _ = _get_nc()
    res = run_bass_kernel_spmd(nc, in_maps, core_ids=list(range(N_CORES)))
    return assemble_output(res.results)



# revision 11
# speedup vs baseline: 1.1256x; 1.1256x over previous
"""Trainium2 Bass kernel for nn_AtomsGPT (GPT-2-style dense transformer).

B=4, T=1024, D=1024, H=16 heads, L=8 layers, V=50257, tied LM head.

Sharding (8 NeuronCores):
- Token-data-parallel trunk: core c owns batch c//2, pair-rank r=c%2.
  Rank r takes the even (r=0) / odd (r=1) 128-position tiles of the
  sequence, interleaved for causal-attention load balance.
- Per layer, the pair exchanges LN1 outputs via a 2-rank AllGather and
  each core computes K/V for all 1024 positions of its batch (the extra
  K/V matmul is cheaper than exchanging K/V and overlaps the collective).
- The tied LM head is TOKEN-parallel: each core computes logits for its
  own 512 tokens over the full (padded) vocab, streaming the embedding
  through SBUF. No final collective at all; the embedding stream and the
  output writes hide behind the head matmuls.

Perf-oriented structure (vs the v1 baseline):
- Attention is software-pipelined with a 1-head skew (QK of head h
  interleaved with AV of head h-1) so the tensor engine never idles and
  the HAM clock gate stays at 2.4 GHz.
- QK scores for the two K-source ranks land in one 2-bank PSUM slot and
  get a single fused exp per (head, k-tile); causal masks are fused
  [128,2,128] gpsimd multiplies.
- Softmax denominators are collected into a [16, 512] tile and
  reciprocal'd ONCE per layer on DVE (was: 8x [128,512] reciprocals),
  then broadcast via tiny PE matmuls against a ones row.
- LayerNorm rstd uses exp(-0.5*ln(var+eps)) so the ACT engine stays on
  the natural_log_exp table set through LN1/attention/LN2 (the only
  table switches per layer are into/out of gelu).
- LN1 of layer l+1 is interleaved tile-wise with FF2 of layer l (and LN2
  with the projection) so vector work hides behind matmuls.
- Head matmuls share each LDWEIGHTS across 4 moving vocab chunks.

All matmuls run in bf16 with fp32 PSUM accumulation; the residual stream
and layernorm statistics stay fp32. LN scales and the attention scale
are folded into weight matrices on the host (exact); all bias vectors in
this problem are structurally zero (asserted).
"""

import sys

for _p in ("/opt/trn_rl_repo", "/root/.axon_site"):
    if _p not in sys.path:
        sys.path.insert(0, _p)

import numpy as np
import ml_dtypes

import concourse.bass as bass
import concourse.tile as tile
from concourse import bacc, mybir
from concourse.bass_utils import run_bass_kernel_spmd

F32 = mybir.dt.float32
BF16 = mybir.dt.bfloat16
AF = mybir.ActivationFunctionType
OP = mybir.AluOpType

B, T, D, H, L, V = 4, 1024, 1024, 16, 8, 50257
HD = D // H  # 64
EPS = 1e-5
N_CORES = 8
TOK = 512           # tokens per core
P = 128
VPT = 51200         # padded vocab for the token-parallel head (25 * 2048)
PAIRS = [[0, 1], [2, 3], [4, 5], [6, 7]]


def positions_for_rank(r):
    """Global positions owned by pair-rank r, in local order (increasing)."""
    tiles = [2 * j + r for j in range(4)]
    return np.concatenate([np.arange(128 * t, 128 * (t + 1)) for t in tiles])


class LNState:
    """Per-LN-instance tiny stat tiles (one [128,4] slot per token tile)."""

    def __init__(self, stat, tagp, name):
        self.ssum = stat.tile([128, 4], F32, tag=f"{tagp}_ssum", name=f"{name}_ssum")
        self.ssq = stat.tile([128, 4], F32, tag=f"{tagp}_ssq", name=f"{name}_ssq")
        self.rstd = stat.tile([128, 4], F32, tag=f"{tagp}_rstd", name=f"{name}_rstd")
        self.nmr = stat.tile([128, 4], F32, tag=f"{tagp}_nmr", name=f"{name}_nmr")


def ln_tile(nc, sb, psT, st, x_ap, t, xn_fm, ident, evac_eng, eps):
    """LayerNorm of token tile t: x_ap[:, t, :] (token-major f32 [128,1024])
    -> feature-major bf16 columns xn_fm[:, :, t*128:(t+1)*128].
    rstd computed as exp(-0.5*ln(var+eps)) to stay in the ln/exp ACT table
    set. Scale/bias are folded into downstream weights on the host."""
    nc.vector.reduce_sum(st.ssum[:, t:t + 1], x_ap[:, t, :], axis=mybir.AxisListType.X)
    sc = sb.tile([128, 1024], F32, tag="ln_sc", name="ln_sc", bufs=1)
    nc.scalar.activation(sc[:], x_ap[:, t, :], AF.Square,
                         accum_out=st.ssq[:, t:t + 1])
    m = sb.tile([128, 1], F32, tag="ln_m", name="ln_m")
    nc.vector.tensor_scalar_mul(m[:], st.ssum[:, t:t + 1], 1.0 / D)
    var = sb.tile([128, 1], F32, tag="ln_var", name="ln_var")
    nc.vector.tensor_scalar_mul(var[:], st.ssq[:, t:t + 1], 1.0 / D)
    mm_ = sb.tile([128, 1], F32, tag="ln_mm", name="ln_mm")
    nc.vector.tensor_mul(mm_[:], m[:], m[:])
    nc.vector.tensor_sub(var[:], var[:], mm_[:])
    # rstd = exp(-0.5 * ln(var + eps)); ln & exp share one ACT table set
    lnv = sb.tile([128, 1], F32, tag="ln_lnv", name="ln_lnv")
    nc.scalar.activation(lnv[:], var[:], AF.Ln, bias=eps[:])
    nc.scalar.activation(st.rstd[:, t:t + 1], lnv[:], AF.Exp, scale=-0.5)
    nc.vector.tensor_mul(st.nmr[:, t:t + 1], m[:], st.rstd[:, t:t + 1])
    nc.vector.tensor_scalar_mul(st.nmr[:, t:t + 1], st.nmr[:, t:t + 1], -1.0)
    xn = sb.tile([128, 1024], BF16, tag="ln_xn", name="ln_xn")
    nc.vector.tensor_scalar(xn[:], x_ap[:, t, :], st.rstd[:, t:t + 1],
                            st.nmr[:, t:t + 1], OP.mult, OP.add)
    ptr = psT.tile([128, 8, 128], BF16, tag="tr", name="ln_tr")
    for kk in range(8):
        nc.tensor.transpose(ptr[:, kk, :], xn[:, kk * 128:(kk + 1) * 128], ident[:])
    if evac_eng == 0:
        nc.vector.tensor_copy(xn_fm[:, :, t * 128:(t + 1) * 128], ptr[:])
    else:
        nc.scalar.copy(xn_fm[:, :, t * 128:(t + 1) * 128], ptr[:])


def build(n_layers=L, dbg=False, no_cc=False, stages=99):
    nc = bacc.Bacc("TRN2", target_bir_lowering=False, debug=False,
                   num_devices=N_CORES)

    x0_h = nc.dram_tensor("x0", [TOK, D], F32, kind="ExternalInput")
    wqkv_h = nc.dram_tensor("wqkv", [n_layers, D, 3 * D], BF16, kind="ExternalInput")
    wp_h = nc.dram_tensor("wp", [n_layers, D, D], BF16, kind="ExternalInput")
    w1_h = nc.dram_tensor("w1", [n_layers, D, 4 * D], BF16, kind="ExternalInput")
    w2_h = nc.dram_tensor("w2", [n_layers, 4 * D, D], BF16, kind="ExternalInput")
    embT_h = nc.dram_tensor("embT", [D, VPT], BF16, kind="ExternalInput")
    msk_h = nc.dram_tensor("msk", [2, 128, 128], BF16, kind="ExternalInput")
    ident_h = nc.dram_tensor("identin", [128, 128], BF16, kind="ExternalInput")
    ones64_h = nc.dram_tensor("ones64", [1, 64], BF16, kind="ExternalInput")
    # token-parallel head output: this core's 512 tokens x padded vocab
    out_h = nc.dram_tensor("out", [TOK, VPT], F32, kind="ExternalOutput")

    dbg_outs = {}

    def dbg_dump(name, ap, shape, rearr=None):
        if not dbg:
            return
        t = nc.dram_tensor(f"dbg_{name}", list(shape), ap.dtype, kind="ExternalOutput")
        dst = t.ap() if rearr is None else t.ap().rearrange(rearr)
        nc.sync.dma_start(dst, ap)
        dbg_outs[name] = shape

    ag_in = [nc.dram_tensor(f"agin{l}", [D, TOK], BF16, kind="Internal")
             for l in range(n_layers)]
    ag_out = [nc.dram_tensor(f"agout{l}", [2 * D, TOK], BF16, kind="Internal")
              for l in range(n_layers)]

    with tile.TileContext(nc) as tc:
      with tc.tile_pool(name="const", bufs=1) as constp, \
           tc.tile_pool(name="xres", bufs=1) as xresp:
        ident = constp.tile([128, 128], BF16)
        nc.sync.dma_start(ident[:], ident_h[:])
        msk = constp.tile([128, 2, 128], BF16)
        nc.sync.dma_start(msk[:], msk_h.ap().rearrange("b p q -> p b q"))
        ones64 = constp.tile([1, 64], BF16)
        nc.sync.dma_start(ones64[:], ones64_h[:])
        eps = constp.tile([128, 1], F32)
        nc.vector.memset(eps[:], EPS)

        # final-LN output lives across the trunk/head scope boundary
        xnf_fm = xresp.tile([128, 8, TOK], BF16)

        with tc.tile_pool(name="stat", bufs=2) as stat, \
             tc.tile_pool(name="sb", bufs=2) as sb, \
             tc.tile_pool(name="act", bufs=1) as actp, \
             tc.tile_pool(name="wch", bufs=4) as wch, \
             tc.tile_pool(name="pp", bufs=8) as pp, \
             tc.tile_pool(name="psMM", bufs=2, space="PSUM") as psMM, \
             tc.tile_pool(name="psAV", bufs=3, space="PSUM") as psAV, \
             tc.tile_pool(name="psT", bufs=1, space="PSUM") as psT:

            # residual stream, token-major fp32 [part, tok-tile, D]
            x = xresp.tile([128, 4, D], F32)
            nc.sync.dma_start(x[:], x0_h.ap().rearrange("(t p) d -> p t d", p=128))

            # layer-0 LN1 (not overlapped with anything)
            st0 = LNState(stat, "l1", "st0")
            xn_fm = actp.tile([128, 8, TOK], BF16, tag="xn_fm", name="xn_fm0")
            for t in range(4):
                ln_tile(nc, sb, psT, st0, x, t, xn_fm, ident, t % 2, eps)

            for l in range(n_layers):
                # kick the pair AllGather of LN1 output
                nc.sync.dma_start(
                    ag_in[l].ap().rearrange("(kk p) t -> p kk t", p=128), xn_fm[:])
                if no_cc:
                    nc.sync.dma_start(ag_out[l][0:D, :], ag_in[l][:])
                    nc.sync.dma_start(ag_out[l][D:2 * D, :], ag_in[l][:])
                else:
                    nc.gpsimd.collective_compute(
                        "AllGather", OP.bypass, replica_groups=PAIRS,
                        ins=[ag_in[l][:]], outs=[ag_out[l][:]])
                if l == 0:
                    dbg_dump("xn_fm0", xn_fm[:], [128, 8, TOK])

                # ---- Q^T (feature-major) from local xn_fm; overlaps the AG
                if stages < 3:
                    continue
                q_fm = actp.tile([128, 8, TOK], BF16, tag="q_fm", name="q_fm")
                for ch in range(2):
                    wt = wch.tile([128, 8, 512], BF16, tag="w", name=f"wq{l}_{ch}")
                    nc.sync.dma_start(
                        wt[:], wqkv_h[l, :, ch * 512:(ch + 1) * 512].rearrange(
                            "(kk p) c -> p kk c", p=128))
                    for m2 in range(2):
                        ps = psMM.tile([128, 2, 512], F32, tag="mm", name="qps")
                        for kk in range(8):
                            nc.tensor.matmul(
                                ps[:, 0, :], wt[:, kk, m2 * 256:m2 * 256 + 128],
                                xn_fm[:, kk, :], start=(kk == 0), stop=(kk == 7))
                            nc.tensor.matmul(
                                ps[:, 1, :], wt[:, kk, m2 * 256 + 128:m2 * 256 + 256],
                                xn_fm[:, kk, :], start=(kk == 0), stop=(kk == 7))
                        dst = q_fm[:, ch * 4 + m2 * 2:ch * 4 + m2 * 2 + 2, :]
                        if m2 == 0:
                            nc.scalar.copy(dst, ps[:])
                        else:
                            nc.vector.tensor_copy(dst, ps[:])

                # ---- gathered xn (both ranks) from the AllGather
                if stages < 4:
                    continue
                xn_src = actp.tile([128, 16, TOK], BF16, tag="h_sb", name="xn_src")
                nc.sync.dma_start(
                    xn_src[:],
                    ag_out[l].ap().rearrange("(b kk p) t -> p (b kk) t", b=2, p=128))

                # ---- K^T (feature-major) for all 1024 positions
                k_all = actp.tile([128, 16, TOK], BF16, tag="k_all", name="k_all")
                for ch in range(2):
                    wt = wch.tile([128, 8, 512], BF16, tag="w", name=f"wk{l}_{ch}")
                    nc.sync.dma_start(
                        wt[:], wqkv_h[l, :, D + ch * 512:D + (ch + 1) * 512].rearrange(
                            "(kk p) c -> p kk c", p=128))
                    for b in range(2):
                        for m2 in range(2):
                            ps = psMM.tile([128, 2, 512], F32, tag="mm", name="kps")
                            for kk in range(8):
                                nc.tensor.matmul(
                                    ps[:, 0, :], wt[:, kk, m2 * 256:m2 * 256 + 128],
                                    xn_src[:, b * 8 + kk, :], start=(kk == 0), stop=(kk == 7))
                                nc.tensor.matmul(
                                    ps[:, 1, :], wt[:, kk, m2 * 256 + 128:m2 * 256 + 256],
                                    xn_src[:, b * 8 + kk, :], start=(kk == 0), stop=(kk == 7))
                            dst = k_all[:, b * 8 + ch * 4 + m2 * 2:
                                        b * 8 + ch * 4 + m2 * 2 + 2, :]
                            if (b + m2) % 2 == 0:
                                nc.vector.tensor_copy(dst, ps[:])
                            else:
                                nc.scalar.copy(dst, ps[:])

                # ---- V (token-major) for all positions, with ones column
                if stages < 5:
                    continue
                v_all = actp.tile([128, 8, H, HD + 1], BF16, tag="v_all", name="v_all")
                nc.vector.memset(v_all[:, :, :, HD:HD + 1], 1.0)
                for ch in range(2):
                    wt = wch.tile([128, 8, 512], BF16, tag="w", name=f"wv{l}_{ch}")
                    nc.sync.dma_start(
                        wt[:], wqkv_h[l, :, 2 * D + ch * 512:2 * D + (ch + 1) * 512].rearrange(
                            "(kk p) c -> p kk c", p=128))
                    for b in range(2):
                        for t2 in range(2):
                            ps = psMM.tile([128, 2, 512], F32, tag="mm", name="vps")
                            for kk in range(8):
                                nc.tensor.matmul(
                                    ps[:, 0, :],
                                    xn_src[:, b * 8 + kk, t2 * 256:t2 * 256 + 128],
                                    wt[:, kk, :], start=(kk == 0), stop=(kk == 7))
                                nc.tensor.matmul(
                                    ps[:, 1, :],
                                    xn_src[:, b * 8 + kk, t2 * 256 + 128:t2 * 256 + 256],
                                    wt[:, kk, :], start=(kk == 0), stop=(kk == 7))
                            for sub in range(2):
                                t = t2 * 2 + sub
                                dst = v_all[:, b * 4 + t, ch * 8:(ch + 1) * 8, 0:HD]
                                src = ps[:, sub, :].rearrange("p (h d) -> p h d", h=8)
                                if (t + b) % 2 == 0:
                                    nc.vector.tensor_copy(dst, src)
                                else:
                                    nc.scalar.copy(dst, src)
                if l == 0:
                    dbg_dump("k_all0", k_all[:], [128, 16, TOK])
                    dbg_dump("v_all0", v_all[:], [128, 8, H, HD + 1])

                # ---- attention: 1-head-skew software pipeline
                if stages < 6:
                    continue
                o_raw = actp.tile([128, 8, TOK], BF16, tag="xn_fm", name="o_raw")
                o_fm = actp.tile([128, 8, TOK], BF16, tag="o_fm", name="o_fm")
                # engines may only write partition base 0/32/64, so head
                # denominators go to partition-0 pair rows, then a tiny DMA
                # scatters them across partitions for one fast reciprocal.
                den_sp = sb.tile([16, TOK], BF16, tag="den_sp", name="den_sp", bufs=1)
                av_ps = {}
                pt_ts = {}
                den_fp = {}

                def emit_qk(h):
                    po, kt = (h % 2) * 64, h // 2
                    avp = psAV.tile([P, TOK], F32, tag="acc", name=f"av{l}_{h}")
                    av_ps[h] = avp
                    pt_ts[h] = []
                    for i in range(4):
                        n = TOK - 128 * i
                        sp = psMM.tile([128, 2, 512], F32, tag="mm",
                                       name=f"sp{l}_{h}_{i}")
                        for b in range(2):
                            nc.tensor.matmul(
                                sp[:, b, 0:n],
                                k_all[po:po + 64, b * 8 + kt, i * 128:(i + 1) * 128],
                                q_fm[po:po + 64, kt, 128 * i:TOK],
                                start=True, stop=True)
                        pt = pp.tile([128, 2, 512], BF16, tag="p",
                                     name=f"pt{l}_{h}_{i}")
                        pt_ts[h].append(pt)
                        nc.scalar.activation(pt[:, :, 0:n], sp[:, :, 0:n], AF.Exp)
                        nc.gpsimd.tensor_mul(pt[:, :, 0:128], pt[:, :, 0:128], msk[:])

                def emit_av(h):
                    po, kt = (h % 2) * 64, h // 2
                    avp = av_ps[h]
                    for i in range(4):
                        n = TOK - 128 * i
                        pt = pt_ts[h][i]
                        for b in range(2):
                            nc.tensor.matmul(
                                avp[0:HD + 1, 128 * i:TOK],
                                v_all[:, b * 4 + i, h, :], pt[:, b, 0:n],
                                start=(i == 0 and b == 0), stop=(i == 3 and b == 1))
                    # evacuate raw (unnormalized) o and the denominator row
                    nc.vector.tensor_copy(o_raw[po:po + 64, kt, :], avp[0:HD, :])
                    if h % 2 == 0:
                        dfp = pp.tile([1, 2, TOK], BF16, tag="dfp", bufs=2,
                                      name=f"dfp{l}_{h // 2}")
                        den_fp[h // 2] = dfp
                        nc.scalar.copy(dfp[0:1, 0, :], avp[HD:HD + 1, :])
                    else:
                        dfp = den_fp.pop(h // 2)
                        nc.vector.tensor_copy(dfp[0:1, 1, :], avp[HD:HD + 1, :])
                        nc.sync.dma_start(den_sp[h - 1:h + 1, :], dfp[:])
                    del av_ps[h], pt_ts[h]

                for h in range(H + 1):
                    if h < H:
                        emit_qk(h)
                    if h > 0:
                        emit_av(h - 1)

                # one reciprocal for all 16 head denominators, then per-pair
                # flatten back to partition 0 (matmul rhs base) and broadcast
                rden = sb.tile([16, TOK], F32, tag="rden", name="rden", bufs=1)
                nc.vector.reciprocal(rden[:], den_sp[:])
                rden_b = sb.tile([16, TOK], BF16, tag="rden_b", name="rden_b", bufs=1)
                nc.vector.tensor_copy(rden_b[:], rden[:])
                for j in range(H // 2):
                    rfp = pp.tile([1, 2, TOK], BF16, tag="rfp", bufs=2,
                                  name=f"rfp{l}_{j}")
                    nc.sync.dma_start(rfp[:], rden_b[2 * j:2 * j + 2, :])
                    bp = psAV.tile([P, TOK], F32, tag="acc", name=f"bp{l}_{j}")
                    nc.tensor.matmul(bp[0:64, :], ones64[:], rfp[0:1, 0, :],
                                     start=True, stop=True)
                    nc.tensor.matmul(bp[64:128, :], ones64[:], rfp[0:1, 1, :],
                                     start=True, stop=True)
                    nc.vector.tensor_tensor(o_fm[:, j, :], o_raw[:, j, :], bp[:],
                                            OP.mult)
                if l == 0:
                    dbg_dump("o_fm0", o_fm[:], [128, 8, TOK])

                # ---- projection (token-major) + residual; LN2 per tile
                if stages < 7:
                    continue
                st2 = LNState(stat, "l2", f"st2_{l}")
                xn2_fm = actp.tile([128, 8, TOK], BF16, tag="xn2_fm", name="xn2_fm")
                wtp = []
                for ch in range(2):
                    w_ = wch.tile([128, 8, 512], BF16, tag="w", name=f"wpj{l}_{ch}")
                    nc.sync.dma_start(
                        w_[:], wp_h[l, :, ch * 512:(ch + 1) * 512].rearrange(
                            "(kk p) c -> p kk c", p=128))
                    wtp.append(w_)
                for t in range(4):
                    ps = psMM.tile([128, 2, 512], F32, tag="mm", name=f"pjps{t}")
                    for kk in range(8):
                        lt = o_fm[:, kk, t * 128:(t + 1) * 128]
                        nc.tensor.matmul(ps[:, 0, :], lt, wtp[0][:, kk, :],
                                         start=(kk == 0), stop=(kk == 7))
                        nc.tensor.matmul(ps[:, 1, :], lt, wtp[1][:, kk, :],
                                         start=(kk == 0), stop=(kk == 7))
                    nc.vector.tensor_add(x[:, t, :], x[:, t, :],
                                         ps[:].rearrange("p b c -> p (b c)"))
                    ln_tile(nc, sb, psT, st2, x, t, xn2_fm, ident, t % 2, eps)
                if l == 0:
                    dbg_dump("xattn0", x[:], [128, 4, D])

                # ---- FFN
                if stages < 8:
                    continue
                h_sb = actp.tile([128, 32, TOK], BF16, tag="h_sb", name="h_sb", bufs=1)
                for mc in range(8):
                    wt = wch.tile([128, 8, 512], BF16, tag="w", name=f"w1_{l}_{mc}")
                    nc.sync.dma_start(
                        wt[:], w1_h[l, :, mc * 512:(mc + 1) * 512].rearrange(
                            "(kk p) c -> p kk c", p=128))
                    for m2 in range(2):
                        ps = psMM.tile([128, 2, 512], F32, tag="mm", name="f1ps")
                        for kk in range(8):
                            nc.tensor.matmul(
                                ps[:, 0, :], wt[:, kk, m2 * 256:m2 * 256 + 128],
                                xn2_fm[:, kk, :], start=(kk == 0), stop=(kk == 7))
                            nc.tensor.matmul(
                                ps[:, 1, :], wt[:, kk, m2 * 256 + 128:m2 * 256 + 256],
                                xn2_fm[:, kk, :], start=(kk == 0), stop=(kk == 7))
                        nc.scalar.activation(
                            h_sb[:, mc * 4 + m2 * 2:mc * 4 + m2 * 2 + 2, :],
                            ps[:], AF.Gelu)

                # FF2 in token-tile pairs; LN1 of the NEXT layer (or the final
                # LN) interleaves tile-wise right after each residual add.
                stn = LNState(stat, "l1", f"stn_{l}")
                if l + 1 < n_layers:
                    xnn_fm = actp.tile([128, 8, TOK], BF16, tag="xn_fm",
                                       name=f"xn_fm{l + 1}")
                else:
                    xnn_fm = xnf_fm
                do_ln = stages >= 9 or l + 1 < n_layers
                for tp in range(2):
                    psf = [psMM.tile([128, 2, 512], F32, tag="mm", name=f"f2ps{tp}_{s}")
                           for s in range(2)]
                    for rg in range(4):
                        w2c = []
                        for hf in range(2):
                            w_ = wch.tile([128, 8, 512], BF16, tag="w",
                                          name=f"w2_{l}_{tp}_{rg}_{hf}")
                            nc.sync.dma_start(
                                w_[:],
                                w2_h[l, rg * 1024:(rg + 1) * 1024,
                                     hf * 512:(hf + 1) * 512].rearrange(
                                    "(kk p) c -> p kk c", p=128))
                            w2c.append(w_)
                        for kki in range(8):
                            kglob = rg * 8 + kki
                            for sub in range(2):
                                t = tp * 2 + sub
                                lt = h_sb[:, kglob, t * 128:(t + 1) * 128]
                                nc.tensor.matmul(
                                    psf[sub][:, 0, :], lt, w2c[0][:, kki, :],
                                    start=(kglob == 0), stop=(kglob == 31))
                                nc.tensor.matmul(
                                    psf[sub][:, 1, :], lt, w2c[1][:, kki, :],
                                    start=(kglob == 0), stop=(kglob == 31))
                    for sub in range(2):
                        t = tp * 2 + sub
                        nc.vector.tensor_add(x[:, t, :], x[:, t, :],
                                             psf[sub][:].rearrange("p b c -> p (b c)"))
                        if do_ln:
                            ln_tile(nc, sb, psT, stn, x, t, xnn_fm, ident, t % 2, eps)
                if l == 0:
                    dbg_dump("xlayer0", x[:], [128, 4, D])
                xn_fm = xnn_fm

            if stages < 9:
                xdump = nc.dram_tensor("xdump", [128, 4, D], F32, kind="ExternalOutput")
                nc.sync.dma_start(xdump.ap(), x[:])
            elif dbg:
                dbg_dump("xnf_fm", xnf_fm[:], [128, 8, TOK])

        # ---- LM head phase: token-parallel over the full padded vocab.
        # embT streams through SBUF in 2048-col quads; each LDWEIGHTS of a
        # 128-token xnf block is shared by 4 moving vocab chunks.
        if stages >= 9:
          with tc.tile_pool(name="embq", bufs=3) as embq, \
               tc.tile_pool(name="hout", bufs=4) as hout, \
               tc.tile_pool(name="psH", bufs=2, space="PSUM") as psH:
            embT_src = embT_h.ap().rearrange("(kk p) v -> p kk v", p=128)
            for vq in range(VPT // 2048):
                et = embq.tile([128, 8, 2048], BF16, tag="e", name=f"et{vq}")
                nc.sync.dma_start(et[:], embT_src[:, :, vq * 2048:(vq + 1) * 2048])
                for tb in range(4):
                    ps = psH.tile([128, 4, 512], F32, tag="h", name=f"hps{vq}_{tb}")
                    for kk in range(8):
                        lt = xnf_fm[:, kk, tb * 128:(tb + 1) * 128]
                        for q4 in range(4):
                            nc.tensor.matmul(
                                ps[:, q4, :], lt, et[:, kk, q4 * 512:(q4 + 1) * 512],
                                start=(kk == 0), stop=(kk == 7))
                    for half in range(2):
                        osb = hout.tile([128, 2, 512], F32, tag="o", name="hosb")
                        if (vq * 4 + tb + half) % 2 == 0:
                            nc.vector.tensor_copy(osb[:], ps[:, half * 2:half * 2 + 2, :])
                        else:
                            nc.scalar.copy(osb[:], ps[:, half * 2:half * 2 + 2, :])
                        nc.sync.dma_start(
                            out_h[tb * 128:(tb + 1) * 128,
                                  vq * 2048 + half * 1024:vq * 2048 + (half + 1) * 1024],
                            osb[:].rearrange("p b c -> p (b c)"))

    nc.compile()
    return nc, dbg_outs


def prepare_inputs(idx, tok_emb, pos_emb, qkv_w, qkv_b, proj_w, proj_b,
                   ff1_w, ff1_b, ff2_w, ff2_b, ln1_s, ln1_b, ln2_s, ln2_b,
                   lnf_s, lnf_b, n_layers=L):
    """Host-side sharding/folding. Returns per-core in_maps."""
    bf = ml_dtypes.bfloat16
    for name, v in (("qkv_b", qkv_b), ("proj_b", proj_b), ("ff1_b", ff1_b),
                    ("ff2_b", ff2_b), ("ln1_b", ln1_b), ("ln2_b", ln2_b),
                    ("lnf_b", lnf_b)):
        assert np.allclose(np.asarray(v), 0.0), f"nonzero {name} not supported"

    idx = np.asarray(idx)
    tok_emb = np.asarray(tok_emb, np.float32)
    pos_emb = np.asarray(pos_emb, np.float32)
    scale = 1.0 / np.sqrt(HD)

    # fold LN scales + attention scale into weights (exact)
    wqkv = (np.asarray(qkv_w[:n_layers], np.float32)
            * np.asarray(ln1_s[:n_layers], np.float32)[:, :, None]).copy()
    wqkv[:, :, :D] *= scale
    w1 = (np.asarray(ff1_w[:n_layers], np.float32)
          * np.asarray(ln2_s[:n_layers], np.float32)[:, :, None])
    wp = np.asarray(proj_w[:n_layers], np.float32)
    w2 = np.asarray(ff2_w[:n_layers], np.float32)
    embT_pad = np.zeros((D, VPT), np.float32)
    embT_pad[:, :V] = (tok_emb * np.asarray(lnf_s, np.float32)[None, :]).T

    wqkv_b = wqkv.astype(bf)
    wp_b = wp.astype(bf)
    w1_b = w1.astype(bf)
    w2_b = w2.astype(bf)
    embT_b = embT_pad.astype(bf)
    ident = np.eye(128, dtype=bf)
    ones64 = np.ones((1, 64), bf)

    tri = np.tril(np.ones((128, 128), np.float32)).T  # [kt, q] valid kt<=q
    msk_r = [np.zeros((2, 128, 128), np.float32) for _ in range(2)]
    msk_r[0][0] = tri          # even block diag: triangular
    msk_r[0][1] = 0.0          # odd block diag: fully masked
    msk_r[1][0] = 1.0          # even block diag: fully visible
    msk_r[1][1] = tri          # odd block diag: triangular
    msk_b = [m.astype(bf) for m in msk_r]

    in_maps = []
    for c in range(N_CORES):
        b, r = c // 2, c % 2
        pos = positions_for_rank(r)
        x0 = tok_emb[idx[b, pos]] + pos_emb[pos]
        in_maps.append({
            "x0": np.ascontiguousarray(x0, np.float32),
            "wqkv": wqkv_b, "wp": wp_b, "w1": w1_b, "w2": w2_b,
            "embT": embT_b,
            "msk": msk_b[r],
            "identin": ident,
            "ones64": ones64,
        })
    return in_maps


def assemble_output(results):
    """Per-core token-major [512, VPT] f32 -> full logits [B, T, V] f32."""
    logits = np.empty((B, T, V), np.float32)
    pos_r = [positions_for_rank(0), positions_for_rank(1)]
    for c in range(N_CORES):
        bb, rr = c // 2, c % 2
        logits[bb, pos_r[rr], :] = results[c]["out"][:, :V]
    return logits


_NC_CACHE = {}


def _get_nc(n_layers=L, dbg=False):
    key = (n_layers, dbg)
    if key not in _NC_CACHE:
        _NC_CACHE[key] = build(n_layers=n_layers, dbg=dbg)
    return _NC_CACHE[key]


def kernel(**inputs):
    in_maps = prepare_inputs(**inputs)
    nc, _ = _get_nc()
    res = run_bass_kernel_spmd(nc, in_maps, core_ids=list(range(N_CORES)))
    return assemble_output(res.results)


# revision 14
# speedup vs baseline: 1.2493x; 1.1099x over previous
"""Trainium2 Bass kernel for nn_AtomsGPT (GPT-2-style dense transformer).

B=4, T=1024, D=1024, H=16 heads, L=8 layers, V=50257, tied LM head.

Sharding (8 NeuronCores):
- Token-data-parallel trunk: core c owns batch c//2, pair-rank r=c%2.
  Rank r takes the even (r=0) / odd (r=1) 128-position tiles of the
  sequence, interleaved for causal-attention load balance.
- Per layer, the pair exchanges LN1 outputs via a 2-rank AllGather and
  each core computes K/V for all 1024 positions of its batch (the extra
  K/V matmul is cheaper than exchanging K/V and overlaps the collective).
- The tied LM head is TOKEN-parallel: each core computes logits for its
  own 512 tokens over the full (padded) vocab, streaming the embedding
  through SBUF. No final collective at all; the embedding stream and the
  output writes hide behind the head matmuls.

Perf-oriented structure (vs the v1 baseline):
- Attention is software-pipelined with a 1-head skew (QK of head h
  interleaved with AV of head h-1) so the tensor engine never idles and
  the HAM clock gate stays at 2.4 GHz.
- QK scores for the two K-source ranks land in one 2-bank PSUM slot and
  get a single fused exp per (head, k-tile); causal masks are fused
  [128,2,128] gpsimd multiplies.
- Softmax denominators are collected into a [16, 512] tile and
  reciprocal'd ONCE per layer on DVE (was: 8x [128,512] reciprocals),
  then broadcast via tiny PE matmuls against a ones row.
- LayerNorm rstd uses exp(-0.5*ln(var+eps)) so the ACT engine stays on
  the natural_log_exp table set through LN1/attention/LN2 (the only
  table switches per layer are into/out of gelu).
- LN1 of layer l+1 is interleaved tile-wise with FF2 of layer l (and LN2
  with the projection) so vector work hides behind matmuls.
- Head matmuls share each LDWEIGHTS across 4 moving vocab chunks.

All matmuls run in bf16 with fp32 PSUM accumulation; the residual stream
and layernorm statistics stay fp32. LN scales and the attention scale
are folded into weight matrices on the host (exact); all bias vectors in
this problem are structurally zero (asserted).
"""

import sys

for _p in ("/opt/trn_rl_repo", "/root/.axon_site"):
    if _p not in sys.path:
        sys.path.insert(0, _p)

import numpy as np
import ml_dtypes

import concourse.bass as bass
import concourse.tile as tile
from concourse import bacc, mybir
from concourse.bass_utils import run_bass_kernel_spmd

F32 = mybir.dt.float32
BF16 = mybir.dt.bfloat16
AF = mybir.ActivationFunctionType
OP = mybir.AluOpType

B, T, D, H, L, V = 4, 1024, 1024, 16, 8, 50257
HD = D // H  # 64
EPS = 1e-5
N_CORES = 8
TOK = 512           # tokens per core
P = 128
VPT = 51200         # padded vocab for the token-parallel head (25 * 2048)
PAIRS = [[0, 1], [2, 3], [4, 5], [6, 7]]


def positions_for_rank(r):
    """Global positions owned by pair-rank r, in local order (increasing)."""
    tiles = [2 * j + r for j in range(4)]
    return np.concatenate([np.arange(128 * t, 128 * (t + 1)) for t in tiles])


class LNState:
    """Per-LN-instance tiny stat tiles (one [128,4] slot per token tile)."""

    def __init__(self, stat, tagp, name):
        self.ssum = stat.tile([128, 4], F32, tag=f"{tagp}_ssum", name=f"{name}_ssum")
        self.ssq = stat.tile([128, 4], F32, tag=f"{tagp}_ssq", name=f"{name}_ssq")
        self.rstd = stat.tile([128, 4], F32, tag=f"{tagp}_rstd", name=f"{name}_rstd")
        self.nmr = stat.tile([128, 4], F32, tag=f"{tagp}_nmr", name=f"{name}_nmr")


def ln_tile(nc, sb, psT, st, x_ap, t, xn_fm, ident, evac_eng, eps, ag_dst=None):
    """LayerNorm of token tile t: x_ap[:, t, :] (token-major f32 [128,1024])
    -> feature-major bf16 columns xn_fm[:, :, t*128:(t+1)*128].
    rstd computed as exp(-0.5*ln(var+eps)) to stay in the ln/exp ACT table
    set. Scale/bias are folded into downstream weights on the host."""
    nc.vector.reduce_sum(st.ssum[:, t:t + 1], x_ap[:, t, :], axis=mybir.AxisListType.X)
    sc = sb.tile([128, 1024], F32, tag="ln_sc", name="ln_sc", bufs=1)
    nc.scalar.activation(sc[:], x_ap[:, t, :], AF.Square,
                         accum_out=st.ssq[:, t:t + 1])
    m = sb.tile([128, 1], F32, tag="ln_m", name="ln_m")
    nc.vector.tensor_scalar_mul(m[:], st.ssum[:, t:t + 1], 1.0 / D)
    var = sb.tile([128, 1], F32, tag="ln_var", name="ln_var")
    nc.vector.tensor_scalar_mul(var[:], st.ssq[:, t:t + 1], 1.0 / D)
    mm_ = sb.tile([128, 1], F32, tag="ln_mm", name="ln_mm")
    nc.vector.tensor_mul(mm_[:], m[:], m[:])
    nc.vector.tensor_sub(var[:], var[:], mm_[:])
    std = sb.tile([128, 1], F32, tag="ln_std", name="ln_std")
    nc.scalar.activation(std[:], var[:], AF.Sqrt, bias=eps[:])
    nc.vector.reciprocal(st.rstd[:, t:t + 1], std[:])
    nc.vector.tensor_mul(st.nmr[:, t:t + 1], m[:], st.rstd[:, t:t + 1])
    nc.vector.tensor_scalar_mul(st.nmr[:, t:t + 1], st.nmr[:, t:t + 1], -1.0)
    xn = sb.tile([128, 1024], BF16, tag="ln_xn", name="ln_xn")
    nc.vector.tensor_scalar(xn[:], x_ap[:, t, :], st.rstd[:, t:t + 1],
                            st.nmr[:, t:t + 1], OP.mult, OP.add)
    ptr = psT.tile([128, 8, 128], BF16, tag="tr", name="ln_tr")
    for kk in range(8):
        nc.tensor.transpose(ptr[:, kk, :], xn[:, kk * 128:(kk + 1) * 128], ident[:])
    if evac_eng == 0:
        nc.vector.tensor_copy(xn_fm[:, :, t * 128:(t + 1) * 128], ptr[:])
    else:
        nc.scalar.copy(xn_fm[:, :, t * 128:(t + 1) * 128], ptr[:])
    if ag_dst is not None:
        nc.sync.dma_start(ag_dst[:, :, t * 128:(t + 1) * 128],
                          xn_fm[:, :, t * 128:(t + 1) * 128])


def build(n_layers=L, dbg=False, no_cc=False, stages=99):
    nc = bacc.Bacc("TRN2", target_bir_lowering=False, debug=False,
                   num_devices=N_CORES)

    x0_h = nc.dram_tensor("x0", [TOK, D], F32, kind="ExternalInput")
    wqkv_h = nc.dram_tensor("wqkv", [n_layers, D, 3 * D], BF16, kind="ExternalInput")
    wp_h = nc.dram_tensor("wp", [n_layers, D, D], BF16, kind="ExternalInput")
    w1_h = nc.dram_tensor("w1", [n_layers, D, 4 * D], BF16, kind="ExternalInput")
    w2_h = nc.dram_tensor("w2", [n_layers, 4 * D, D], BF16, kind="ExternalInput")
    embT_h = nc.dram_tensor("embT", [D, VPT], BF16, kind="ExternalInput")
    msk_h = nc.dram_tensor("msk", [2, 128, 128], BF16, kind="ExternalInput")
    ident_h = nc.dram_tensor("identin", [128, 128], BF16, kind="ExternalInput")
    ones64_h = nc.dram_tensor("ones64", [1, 64], BF16, kind="ExternalInput")
    # token-parallel head output: this core's 512 tokens x padded vocab
    out_h = nc.dram_tensor("out", [TOK, VPT], F32, kind="ExternalOutput")

    dbg_outs = {}

    def dbg_dump(name, ap, shape, rearr=None):
        if not dbg:
            return
        t = nc.dram_tensor(f"dbg_{name}", list(shape), ap.dtype, kind="ExternalOutput")
        dst = t.ap() if rearr is None else t.ap().rearrange(rearr)
        nc.sync.dma_start(dst, ap)
        dbg_outs[name] = shape

    ag_in = [nc.dram_tensor(f"agin{l}", [D, TOK], BF16, kind="Internal")
             for l in range(n_layers)]
    ag_out = [nc.dram_tensor(f"agout{l}", [2 * D, TOK], BF16, kind="Internal")
              for l in range(n_layers)]

    with tile.TileContext(nc) as tc:
      with tc.tile_pool(name="const", bufs=1) as constp, \
           tc.tile_pool(name="xres", bufs=1) as xresp:
        ident = constp.tile([128, 128], BF16)
        nc.sync.dma_start(ident[:], ident_h[:])
        msk = constp.tile([128, 2, 128], BF16)
        nc.sync.dma_start(msk[:], msk_h.ap().rearrange("b p q -> p b q"))
        ones64 = constp.tile([1, 64], BF16)
        nc.sync.dma_start(ones64[:], ones64_h[:])
        eps = constp.tile([128, 1], F32)
        nc.vector.memset(eps[:], EPS)

        # final-LN output lives across the trunk/head scope boundary
        xnf_fm = xresp.tile([128, 8, TOK], BF16)

        with tc.tile_pool(name="stat", bufs=2) as stat, \
             tc.tile_pool(name="sb", bufs=2) as sb, \
             tc.tile_pool(name="act", bufs=1) as actp, \
             tc.tile_pool(name="wch", bufs=5) as wch, \
             tc.tile_pool(name="pp", bufs=8) as pp, \
             tc.tile_pool(name="psMM", bufs=2, space="PSUM") as psMM, \
             tc.tile_pool(name="psAV", bufs=3, space="PSUM") as psAV, \
             tc.tile_pool(name="psT", bufs=1, space="PSUM") as psT:

            # residual stream, token-major fp32 [part, tok-tile, D]
            x = xresp.tile([128, 4, D], F32)
            nc.sync.dma_start(x[:], x0_h.ap().rearrange("(t p) d -> p t d", p=128))

            # layer-0 LN1 (not overlapped with anything)
            st0 = LNState(stat, "l1", "st0")
            xn_fm = actp.tile([128, 8, TOK], BF16, tag="xn_fm", name="xn_fm0")
            ag0_dst = ag_in[0].ap().rearrange("(kk p) t -> p kk t", p=128)
            for t in range(4):
                ln_tile(nc, sb, psT, st0, x, t, xn_fm, ident, t % 2, eps,
                        ag_dst=ag0_dst)

            for l in range(n_layers):
                # the pair AllGather of LN1 output (ag_in streamed per-tile)
                if no_cc:
                    nc.sync.dma_start(ag_out[l][0:D, :], ag_in[l][:])
                    nc.sync.dma_start(ag_out[l][D:2 * D, :], ag_in[l][:])
                else:
                    nc.gpsimd.collective_compute(
                        "AllGather", OP.bypass, replica_groups=PAIRS,
                        ins=[ag_in[l][:]], outs=[ag_out[l][:]])
                if l == 0:
                    dbg_dump("xn_fm0", xn_fm[:], [128, 8, TOK])

                # ---- Q^T (feature-major) from local xn_fm; overlaps the AG
                if stages < 3:
                    continue
                q_fm = actp.tile([128, 8, TOK], BF16, tag="q_fm", name="q_fm")
                for ch in range(2):
                    wt = wch.tile([128, 8, 512], BF16, tag="w", name=f"wq{l}_{ch}")
                    nc.sync.dma_start(
                        wt[:], wqkv_h[l, :, ch * 512:(ch + 1) * 512].rearrange(
                            "(kk p) c -> p kk c", p=128))
                    for m2 in range(2):
                        ps = psMM.tile([128, 2, 512], F32, tag="mm", name="qps")
                        for kk in range(8):
                            nc.tensor.matmul(
                                ps[:, 0, :], wt[:, kk, m2 * 256:m2 * 256 + 128],
                                xn_fm[:, kk, :], start=(kk == 0), stop=(kk == 7))
                            nc.tensor.matmul(
                                ps[:, 1, :], wt[:, kk, m2 * 256 + 128:m2 * 256 + 256],
                                xn_fm[:, kk, :], start=(kk == 0), stop=(kk == 7))
                        dst = q_fm[:, ch * 4 + m2 * 2:ch * 4 + m2 * 2 + 2, :]
                        if m2 == 0:
                            nc.scalar.copy(dst, ps[:])
                        else:
                            nc.vector.tensor_copy(dst, ps[:])

                # ---- gathered xn (both ranks) from the AllGather
                if stages < 4:
                    continue
                xn_src = actp.tile([128, 16, TOK], BF16, tag="h_sb", name="xn_src")
                xa = ag_out[l].ap().rearrange("(b kk p) t -> p (b kk) t", b=2, p=128)
                nc.sync.dma_start(xn_src[:, 0:8, :], xa[:, 0:8, :])
                nc.sync.dma_start(xn_src[:, 8:16, :], xa[:, 8:16, :])

                # weights for K and V (prefetched; reused by the fused stream)
                wtk, wtv = [], []
                for ch in range(2):
                    wk_ = wch.tile([128, 8, 512], BF16, tag="w", name=f"wk{l}_{ch}")
                    nc.sync.dma_start(
                        wk_[:], wqkv_h[l, :, D + ch * 512:D + (ch + 1) * 512].rearrange(
                            "(kk p) c -> p kk c", p=128))
                    wtk.append(wk_)
                for ch in range(2):
                    wv_ = wch.tile([128, 8, 512], BF16, tag="w", name=f"wv{l}_{ch}")
                    nc.sync.dma_start(
                        wv_[:], wqkv_h[l, :, 2 * D + ch * 512:2 * D + (ch + 1) * 512].rearrange(
                            "(kk p) c -> p kk c", p=128))
                    wtv.append(wv_)

                if stages < 5:
                    continue
                k_all = actp.tile([128, 16, TOK], BF16, tag="k_all", name="k_all")
                v_all = actp.tile([128, 8, H, HD + 1], BF16, tag="v_all", name="v_all")
                nc.vector.memset(v_all[:, :, :, HD:HD + 1], 1.0)
                o_raw = actp.tile([128, 8, TOK], BF16, tag="xn_fm", name="o_raw")
                o_fm = actp.tile([128, 8, TOK], BF16, tag="o_fm", name="o_fm")
                den_sp = sb.tile([16, TOK], BF16, tag="den_sp", name="den_sp", bufs=1)
                av_ps = {}
                pt_ts = {}
                den_fp = {}
                nkv = [0]

                def emit_k(j):
                    """K chunk j (heads 2j, 2j+1): two 8-MM chains, fused evac."""
                    ch, sub = j // 4, j % 4
                    ps = psMM.tile([128, 2, 512], F32, tag="mm", name=f"kps{l}_{j}")
                    for b in range(2):
                        for kk in range(8):
                            nc.tensor.matmul(
                                ps[:, b, :], wtk[ch][:, kk, sub * 128:(sub + 1) * 128],
                                xn_src[:, b * 8 + kk, :], start=(kk == 0), stop=(kk == 7))
                    dst = k_all[:].rearrange("p (b j) t -> p b j t", b=2)[:, :, j, :]
                    if j % 2 == 0:
                        nc.vector.tensor_copy(dst, ps[:])
                    else:
                        nc.scalar.copy(dst, ps[:])

                def emit_v(ch, b, t):
                    """V chain for (feature-half ch, source-rank b, token tile t)."""
                    ps = psT.tile([128, 512], F32, tag="tr", name=f"vps{l}_{ch}_{b}_{t}")
                    for kk in range(8):
                        nc.tensor.matmul(
                            ps[:], xn_src[:, b * 8 + kk, t * 128:(t + 1) * 128],
                            wtv[ch][:, kk, :], start=(kk == 0), stop=(kk == 7))
                    dst = v_all[:, b * 4 + t, ch * 8:(ch + 1) * 8, 0:HD]
                    src = ps[:].rearrange("p (h d) -> p h d", h=8)
                    if (nkv[0] % 2) == 0:
                        nc.vector.tensor_copy(dst, src)
                    else:
                        nc.scalar.copy(dst, src)
                    nkv[0] += 1

                def emit_qk(h):
                    po, kt = (h % 2) * 64, h // 2
                    avp = psAV.tile([P, TOK], F32, tag="acc", name=f"av{l}_{h}")
                    av_ps[h] = avp
                    pt_ts[h] = []
                    for i in range(4):
                        n = TOK - 128 * i
                        sp = psMM.tile([128, 2, 512], F32, tag="mm",
                                       name=f"sp{l}_{h}_{i}")
                        for b in range(2):
                            nc.tensor.matmul(
                                sp[:, b, 0:n],
                                k_all[po:po + 64, b * 8 + kt, i * 128:(i + 1) * 128],
                                q_fm[po:po + 64, kt, 128 * i:TOK],
                                start=True, stop=True)
                        pt = pp.tile([128, 2, 512], BF16, tag="p",
                                     name=f"pt{l}_{h}_{i}")
                        pt_ts[h].append(pt)
                        nc.scalar.activation(pt[:, :, 0:n], sp[:, :, 0:n], AF.Exp)
                        nc.gpsimd.tensor_mul(pt[:, :, 0:128], pt[:, :, 0:128], msk[:])

                def emit_av(h):
                    po, kt = (h % 2) * 64, h // 2
                    avp = av_ps[h]
                    for i in range(4):
                        n = TOK - 128 * i
                        pt = pt_ts[h][i]
                        for b in range(2):
                            nc.tensor.matmul(
                                avp[0:HD + 1, 128 * i:TOK],
                                v_all[:, b * 4 + i, h, :], pt[:, b, 0:n],
                                start=(i == 0 and b == 0), stop=(i == 3 and b == 1))
                    nc.vector.tensor_copy(o_raw[po:po + 64, kt, :], avp[0:HD, :])
                    if h % 2 == 0:
                        dfp = pp.tile([1, 2, TOK], BF16, tag="dfp", bufs=2,
                                      name=f"dfp{l}_{h // 2}")
                        den_fp[h // 2] = dfp
                        nc.vector.tensor_copy(dfp[0:1, 0, :], avp[HD:HD + 1, :])
                    else:
                        dfp = den_fp[h // 2]
                        nc.vector.tensor_copy(dfp[0:1, 1, :], avp[HD:HD + 1, :])
                        if h < H - 1:
                            nc.sync.dma_start(den_sp[h - 1:h + 1, :], dfp[:])
                            del den_fp[h // 2]
                    del av_ps[h], pt_ts[h]

                def den_batch():
                    """Reciprocal + broadcast + normalize for pairs 0..6."""
                    rden = sb.tile([16, TOK], F32, tag="rden", name="rden", bufs=1)
                    nc.vector.reciprocal(rden[0:14, :], den_sp[0:14, :])
                    rden_b = sb.tile([16, TOK], BF16, tag="rden_b", name="rden_b",
                                     bufs=1)
                    nc.vector.tensor_copy(rden_b[0:14, :], rden[0:14, :])
                    for j in range(7):
                        rfp = pp.tile([1, 2, TOK], BF16, tag="rfp", bufs=2,
                                      name=f"rfp{l}_{j}")
                        nc.sync.dma_start(rfp[:], rden_b[2 * j:2 * j + 2, :])
                        bp = psAV.tile([P, TOK], F32, tag="acc", name=f"bp{l}_{j}")
                        nc.tensor.matmul(bp[0:64, :], ones64[:], rfp[0:1, 0, :],
                                         start=True, stop=True)
                        nc.tensor.matmul(bp[64:128, :], ones64[:], rfp[0:1, 1, :],
                                         start=True, stop=True)
                        nc.vector.tensor_tensor(o_fm[:, j, :], o_raw[:, j, :],
                                                bp[:], OP.mult)

                # fused K/V + attention stream (keeps the PE dense while the
                # ACT engine works through the exps)
                emit_k(0)
                for h in range(H + 1):
                    if h < H:
                        if h % 2 == 0 and h // 2 + 1 < 8:
                            emit_k(h // 2 + 1)
                        if h == 0:
                            for bt in range(8):
                                emit_v(0, bt // 4, bt % 4)
                        if 4 <= h <= 7:
                            emit_v(1, (h - 4) // 2, ((h - 4) % 2) * 2)
                            emit_v(1, (h - 4) // 2, ((h - 4) % 2) * 2 + 1)
                        emit_qk(h)
                    if h > 0:
                        emit_av(h - 1)
                    if h == H - 1:
                        den_batch()

                # last pair (heads 14,15): broadcast the raw denominators and
                # reciprocal on the broadcast so nothing waits on a DMA hop
                dfp7 = den_fp.pop(7)
                bp7 = psAV.tile([P, TOK], F32, tag="acc", name=f"bp{l}_7")
                nc.tensor.matmul(bp7[0:64, :], ones64[:], dfp7[0:1, 0, :],
                                 start=True, stop=True)
                nc.tensor.matmul(bp7[64:128, :], ones64[:], dfp7[0:1, 1, :],
                                 start=True, stop=True)
                rb7 = sb.tile([128, TOK], F32, tag="rb7", name="rb7", bufs=1)
                nc.vector.reciprocal(rb7[:], bp7[:])
                nc.vector.tensor_tensor(o_fm[:, 7, :], o_raw[:, 7, :], rb7[:],
                                        OP.mult)
                if l == 0:
                    dbg_dump("o_fm0", o_fm[:], [128, 8, TOK])

                # ---- projection (token-major) + residual; LN2 per tile
                if stages < 7:
                    continue
                st2 = LNState(stat, "l2", f"st2_{l}")
                xn2_fm = actp.tile([128, 8, TOK], BF16, tag="xn2_fm", name="xn2_fm")
                wtp = []
                for ch in range(2):
                    w_ = wch.tile([128, 8, 512], BF16, tag="w", name=f"wpj{l}_{ch}")
                    nc.sync.dma_start(
                        w_[:], wp_h[l, :, ch * 512:(ch + 1) * 512].rearrange(
                            "(kk p) c -> p kk c", p=128))
                    wtp.append(w_)
                for t2 in range(2):
                    pss = [psMM.tile([128, 2, 512], F32, tag="mm",
                                     name=f"pjps{t2}_{s_}") for s_ in range(2)]
                    for kk in list(range(7)) + [7]:
                        for s_ in range(2):
                            t = t2 * 2 + s_
                            lt = o_fm[:, kk, t * 128:(t + 1) * 128]
                            nc.tensor.matmul(pss[s_][:, 0, :], lt, wtp[0][:, kk, :],
                                             start=(kk == 0), stop=(kk == 7))
                            nc.tensor.matmul(pss[s_][:, 1, :], lt, wtp[1][:, kk, :],
                                             start=(kk == 0), stop=(kk == 7))
                    for s_ in range(2):
                        t = t2 * 2 + s_
                        nc.vector.tensor_add(x[:, t, :], x[:, t, :],
                                             pss[s_][:].rearrange("p b c -> p (b c)"))
                        ln_tile(nc, sb, psT, st2, x, t, xn2_fm, ident, t % 2, eps)
                if l == 0:
                    dbg_dump("xattn0", x[:], [128, 4, D])

                # ---- FFN
                if stages < 8:
                    continue
                h_sb = actp.tile([128, 32, TOK], BF16, tag="h_sb", name="h_sb", bufs=1)
                for mc in range(8):
                    wt = wch.tile([128, 8, 512], BF16, tag="w", name=f"w1_{l}_{mc}")
                    nc.sync.dma_start(
                        wt[:], w1_h[l, :, mc * 512:(mc + 1) * 512].rearrange(
                            "(kk p) c -> p kk c", p=128))
                    for m2 in range(2):
                        ps = psMM.tile([128, 2, 512], F32, tag="mm", name="f1ps")
                        for kk in range(8):
                            nc.tensor.matmul(
                                ps[:, 0, :], wt[:, kk, m2 * 256:m2 * 256 + 128],
                                xn2_fm[:, kk, :], start=(kk == 0), stop=(kk == 7))
                            nc.tensor.matmul(
                                ps[:, 1, :], wt[:, kk, m2 * 256 + 128:m2 * 256 + 256],
                                xn2_fm[:, kk, :], start=(kk == 0), stop=(kk == 7))
                        nc.scalar.activation(
                            h_sb[:, mc * 4 + m2 * 2:mc * 4 + m2 * 2 + 2, :],
                            ps[:], AF.Gelu)

                # FF2 in token-tile pairs; LN1 of the NEXT layer (or the final
                # LN) interleaves tile-wise right after each residual add.
                stn = LNState(stat, "l1", f"stn_{l}")
                if l + 1 < n_layers:
                    xnn_fm = actp.tile([128, 8, TOK], BF16, tag="xn_fm",
                                       name=f"xn_fm{l + 1}")
                else:
                    xnn_fm = xnf_fm
                do_ln = stages >= 9 or l + 1 < n_layers
                for tp in range(2):
                    psf = [psMM.tile([128, 2, 512], F32, tag="mm", name=f"f2ps{tp}_{s}")
                           for s in range(2)]
                    for rg in range(4):
                        w2c = []
                        for hf in range(2):
                            w_ = wch.tile([128, 8, 512], BF16, tag="w",
                                          name=f"w2_{l}_{tp}_{rg}_{hf}")
                            nc.sync.dma_start(
                                w_[:],
                                w2_h[l, rg * 1024:(rg + 1) * 1024,
                                     hf * 512:(hf + 1) * 512].rearrange(
                                    "(kk p) c -> p kk c", p=128))
                            w2c.append(w_)
                        for kki in range(8):
                            kglob = rg * 8 + kki
                            for sub in range(2):
                                t = tp * 2 + sub
                                lt = h_sb[:, kglob, t * 128:(t + 1) * 128]
                                nc.tensor.matmul(
                                    psf[sub][:, 0, :], lt, w2c[0][:, kki, :],
                                    start=(kglob == 0), stop=(kglob == 31))
                                nc.tensor.matmul(
                                    psf[sub][:, 1, :], lt, w2c[1][:, kki, :],
                                    start=(kglob == 0), stop=(kglob == 31))
                    for sub in range(2):
                        t = tp * 2 + sub
                        nc.vector.tensor_add(x[:, t, :], x[:, t, :],
                                             psf[sub][:].rearrange("p b c -> p (b c)"))
                        if do_ln:
                            agd = (ag_in[l + 1].ap().rearrange(
                                       "(kk p) t -> p kk t", p=128)
                                   if l + 1 < n_layers else None)
                            ln_tile(nc, sb, psT, stn, x, t, xnn_fm, ident, t % 2,
                                    eps, ag_dst=agd)
                if l == 0:
                    dbg_dump("xlayer0", x[:], [128, 4, D])
                xn_fm = xnn_fm

            if stages < 9:
                xdump = nc.dram_tensor("xdump", [128, 4, D], F32, kind="ExternalOutput")
                nc.sync.dma_start(xdump.ap(), x[:])
            elif dbg:
                dbg_dump("xnf_fm", xnf_fm[:], [128, 8, TOK])

        # ---- LM head phase: token-parallel over the full padded vocab.
        # embT streams through SBUF in 2048-col quads; each LDWEIGHTS of a
        # 128-token xnf block is shared by 4 moving vocab chunks.
        if stages >= 9:
          with tc.tile_pool(name="embq", bufs=3) as embq, \
               tc.tile_pool(name="hout", bufs=4) as hout, \
               tc.tile_pool(name="psH", bufs=2, space="PSUM") as psH:
            embT_src = embT_h.ap().rearrange("(kk p) v -> p kk v", p=128)
            for vq in range(VPT // 2048):
                et = embq.tile([128, 8, 2048], BF16, tag="e", name=f"et{vq}")
                nc.sync.dma_start(et[:], embT_src[:, :, vq * 2048:(vq + 1) * 2048])
                for tb in range(4):
                    ps = psH.tile([128, 4, 512], F32, tag="h", name=f"hps{vq}_{tb}")
                    for kk in range(8):
                        lt = xnf_fm[:, kk, tb * 128:(tb + 1) * 128]
                        for q4 in range(4):
                            nc.tensor.matmul(
                                ps[:, q4, :], lt, et[:, kk, q4 * 512:(q4 + 1) * 512],
                                start=(kk == 0), stop=(kk == 7))
                    for half in range(2):
                        osb = hout.tile([128, 2, 512], F32, tag="o", name="hosb")
                        if (vq * 4 + tb + half) % 2 == 0:
                            nc.vector.tensor_copy(osb[:], ps[:, half * 2:half * 2 + 2, :])
                        else:
                            nc.scalar.copy(osb[:], ps[:, half * 2:half * 2 + 2, :])
                        nc.sync.dma_start(
                            out_h[tb * 128:(tb + 1) * 128,
                                  vq * 2048 + half * 1024:vq * 2048 + (half + 1) * 1024],
                            osb[:].rearrange("p b c -> p (b c)"))

    nc.compile()
    return nc, dbg_outs


def prepare_inputs(idx, tok_emb, pos_emb, qkv_w, qkv_b, proj_w, proj_b,
                   ff1_w, ff1_b, ff2_w, ff2_b, ln1_s, ln1_b, ln2_s, ln2_b,
                   lnf_s, lnf_b, n_layers=L):
    """Host-side sharding/folding. Returns per-core in_maps."""
    bf = ml_dtypes.bfloat16
    for name, v in (("qkv_b", qkv_b), ("proj_b", proj_b), ("ff1_b", ff1_b),
                    ("ff2_b", ff2_b), ("ln1_b", ln1_b), ("ln2_b", ln2_b),
                    ("lnf_b", lnf_b)):
        assert np.allclose(np.asarray(v), 0.0), f"nonzero {name} not supported"

    idx = np.asarray(idx)
    tok_emb = np.asarray(tok_emb, np.float32)
    pos_emb = np.asarray(pos_emb, np.float32)
    scale = 1.0 / np.sqrt(HD)

    # fold LN scales + attention scale into weights (exact)
    wqkv = (np.asarray(qkv_w[:n_layers], np.float32)
            * np.asarray(ln1_s[:n_layers], np.float32)[:, :, None]).copy()
    wqkv[:, :, :D] *= scale
    w1 = (np.asarray(ff1_w[:n_layers], np.float32)
          * np.asarray(ln2_s[:n_layers], np.float32)[:, :, None])
    wp = np.asarray(proj_w[:n_layers], np.float32)
    w2 = np.asarray(ff2_w[:n_layers], np.float32)
    embT_pad = np.zeros((D, VPT), np.float32)
    embT_pad[:, :V] = (tok_emb * np.asarray(lnf_s, np.float32)[None, :]).T

    wqkv_b = wqkv.astype(bf)
    wp_b = wp.astype(bf)
    w1_b = w1.astype(bf)
    w2_b = w2.astype(bf)
    embT_b = embT_pad.astype(bf)
    ident = np.eye(128, dtype=bf)
    ones64 = np.ones((1, 64), bf)

    tri = np.tril(np.ones((128, 128), np.float32)).T  # [kt, q] valid kt<=q
    msk_r = [np.zeros((2, 128, 128), np.float32) for _ in range(2)]
    msk_r[0][0] = tri          # even block diag: triangular
    msk_r[0][1] = 0.0          # odd block diag: fully masked
    msk_r[1][0] = 1.0          # even block diag: fully visible
    msk_r[1][1] = tri          # odd block diag: triangular
    msk_b = [m.astype(bf) for m in msk_r]

    in_maps = []
    for c in range(N_CORES):
        b, r = c // 2, c % 2
        pos = positions_for_rank(r)
        x0 = tok_emb[idx[b, pos]] + pos_emb[pos]
        in_maps.append({
            "x0": np.ascontiguousarray(x0, np.float32),
            "wqkv": wqkv_b, "wp": wp_b, "w1": w1_b, "w2": w2_b,
            "embT": embT_b,
            "msk": msk_b[r],
            "identin": ident,
            "ones64": ones64,
        })
    return in_maps


def assemble_output(results):
    """Per-core token-major [512, VPT] f32 -> full logits [B, T, V] f32."""
    logits = np.empty((B, T, V), np.float32)
    pos_r = [positions_for_rank(0), positions_for_rank(1)]
    for c in range(N_CORES):
        bb, rr = c // 2, c % 2
        logits[bb, pos_r[rr], :] = results[c]["out"][:, :V]
    return logits


_NC_CACHE = {}


def _get_nc(n_layers=L, dbg=False):
    key = (n_layers, dbg)
    if key not in _NC_CACHE:
        _NC_CACHE[key] = build(n_layers=n_layers, dbg=dbg)
    return _NC_CACHE[key]


def kernel(**inputs):
    in_maps = prepare_inputs(**inputs)
    nc, _ = _get_nc()
    res = run_bass_kernel_spmd(nc, in_maps, core_ids=list(range(N_CORES)))
    return assemble_output(res.results)
